# revision 11
# baseline (speedup 1.0000x reference)
"""Trainium2 Bass kernel for nn_Decoder: preds = hidden @ W + b, loss = sum((preds - outputs.T)^2).

Full-input contract: kernel(**inputs) takes the unsharded inputs and returns
(loss, preds_flat), matching the reference. Internally shards hidden/outputs
along seq_len across 8 NeuronCores (data-parallel), runs one SPMD Bass kernel,
and combines partial results on the host.

Per-core device program (memory-bound: streams its 32 MiB hidden shard once):
  - hidden shard viewed as [128 partitions, 256 rows x 256 floats] so every
    DMA moves 8 KiB contiguous per partition.
  - the 256-long dot products are split across three engines per 8-row tile:
    A_DVE rows fused on DVE (scalar_tensor_tensor + accumulator), the rest
    multiplied on GpSimd, with K_RED rows reduced by one segmented DVE
    tensor_reduce and R_ACT rows by ACT activation+accumulate.
  - loss partials reduced to [128,1] per core on-device; host sums 8x128 values.
"""

import sys

for _p in ("/opt/trn_rl_repo",):
    if _p not in sys.path:
        sys.path.insert(0, _p)

from contextlib import ExitStack

import numpy as np

import concourse.bass as bass
import concourse.tile as tile
from concourse import mybir
from concourse.bass_utils import run_bass_kernel_spmd

S, B, H = 4096, 64, 256
NCORES = 8
S_SH = S // NCORES          # 512 seq positions per core
R = S_SH * B                # 32768 rows per core
P = 128                     # SBUF partitions
N = R // P                  # 256 rows per partition
TJ = 16                     # rows (segments of H) per partition per tile
NT = N // TJ                # 32 tiles per core

# per-tile split of the 8 row-segments across engines. GpSimd is unusable
# here: it shares SBUF ports with DVE, so concurrent gpsimd work ~3x-slows
# DVE ops (measured 422ns -> 1505ns STT).
A_DVE = 8                   # fused dot on DVE (scalar_tensor_tensor)
K_RED = 0                   # (DVE segmented tensor_reduce path - unused)
R_ACT = TJ - A_DVE - K_RED  # DVE batched mult -> ACT activation+accum
WREP = 8                    # W replicated this many times in SBUF (period for slicing)

F32 = mybir.dt.float32


def _split_multiwait_instructions(nc, max_waits=1):
    """This container's walrus build rejects instructions carrying more than
    one sync-wait command ("Too many sync wait commands" in codegen). Tile's
    end-of-context drain waits on every outstanding proc semaphore in a single
    instruction, so split surplus waits into preceding single-wait drains on
    the same engine."""
    for fn in nc.m.functions:
        for bb in fn.blocks:
            il = bb.instructions
            k = 0
            while k < len(il):
                inst = il[k]
                si = getattr(inst, "sync_info", None)
                if si is not None and si.on_wait and len(si.on_wait) > max_waits:
                    waits = list(si.on_wait)
                    si.on_wait = waits[:max_waits]
                    extra = waits[max_waits:]
                    for i in range(0, len(extra), max_waits):
                        d = mybir.InstDrain(
                            name=f"{inst.name}_wsplit{i}", ins=[], outs=[]
                        )
                        d.engine = inst.engine
                        d.sync_info = mybir.SyncInfo(
                            on_wait=extra[i:i + max_waits], on_update=[]
                        )
                        il.insert(k, d)
                        k += 1
                k += 1


def _build_nc():
    nc = bass.Bass()
    xh = nc.declare_dram_parameter("xh", [P, N * H], F32, isOutput=False)
    wvec = nc.declare_dram_parameter("wvec", [1, H], F32, isOutput=False)
    otadj = nc.declare_dram_parameter("otadj", [P, N], F32, isOutput=False)
    brep = nc.declare_dram_parameter("brep", [P, 1], F32, isOutput=False)
    preds_o = nc.declare_dram_parameter("preds_o", [P, N], F32, isOutput=True)
    loss_p = nc.declare_dram_parameter("loss_p", [P, 1], F32, isOutput=True)

    mult = mybir.AluOpType.mult
    subtract = mybir.AluOpType.subtract
    add = mybir.AluOpType.add

    with tile.TileContext(nc) as tc, ExitStack() as ctx:
        singles = ctx.enter_context(tc.tile_pool(name="singles", bufs=1))
        xpool = ctx.enter_context(tc.tile_pool(name="xp", bufs=6))
        prodp = ctx.enter_context(tc.tile_pool(name="pp", bufs=3))
        scrp = ctx.enter_context(tc.tile_pool(name="sc", bufs=2))

        # W broadcast to all partitions, then replicated TJ times along free
        # (doubling copies) so any aligned 256-slice of w_sb is W.
        w_sb = singles.tile([P, WREP * H], F32)
        nc.sync.dma_start(out=w_sb[:, 0:H], in_=wvec[:, :].to_broadcast([P, H]))
        rep = H
        while rep < WREP * H:
            nc.vector.tensor_copy(w_sb[:, rep:2 * rep], w_sb[:, 0:rep])
            rep *= 2

        ot_sb = singles.tile([P, N], F32)
        nc.sync.dma_start(out=ot_sb, in_=otadj[:, :])
        b_sb = singles.tile([P, 1], F32)
        nc.sync.dma_start(out=b_sb, in_=brep[:, :])
        preds_buf = singles.tile([P, N], F32)

        GP = K_RED + R_ACT  # segments multiplied on gpsimd
        for t in range(NT):
            x = xpool.tile([P, TJ * H], F32)
            nc.sync.dma_start(out=x, in_=xh[:, t * TJ * H:(t + 1) * TJ * H])
            c0 = t * TJ
            for j in range(A_DVE):
                sc = scrp.tile([P, H], F32)
                nc.vector.scalar_tensor_tensor(
                    out=sc,
                    in0=x[:, j * H:(j + 1) * H],
                    scalar=1.0,
                    in1=w_sb[:, :H],
                    op0=mult,
                    op1=mult,
                    accum_out=preds_buf[:, c0 + j:c0 + j + 1],
                )
            prod = prodp.tile([P, GP * H], F32)
            # multiply GP segments in WREP-aligned chunks (w_sb holds WREP copies)
            seg = A_DVE
            off = 0
            while seg < TJ:
                cn = min(WREP - (seg % WREP), TJ - seg)
                nc.vector.tensor_mul(
                    prod[:, off * H:(off + cn) * H],
                    x[:, seg * H:(seg + cn) * H],
                    w_sb[:, (seg % WREP) * H:((seg % WREP) + cn) * H],
                )
                seg += cn
                off += cn
            if K_RED:
                nc.vector.tensor_reduce(
                    out=preds_buf[:, c0 + A_DVE:c0 + A_DVE + K_RED],
                    in_=prod[:, 0:K_RED * H].rearrange("p (k h) -> p k h", k=K_RED),
                    axis=mybir.AxisListType.X,
                    op=add,
                )
            for j in range(R_ACT):
                asc = scrp.tile([P, H], F32)
                nc.scalar.activation(
                    out=asc,
                    in_=prod[:, (K_RED + j) * H:(K_RED + j + 1) * H],
                    func=mybir.ActivationFunctionType.Copy,
                    accum_out=preds_buf[:, c0 + A_DVE + K_RED + j:
                                        c0 + A_DVE + K_RED + j + 1],
                )

        d = singles.tile([P, N], F32)
        nc.vector.tensor_tensor(out=d, in0=preds_buf, in1=ot_sb, op=subtract)
        dsc = singles.tile([P, N], F32)
        loss_sb = singles.tile([P, 1], F32)
        nc.vector.scalar_tensor_tensor(
            out=dsc, in0=d, scalar=1.0, in1=d,
            op0=mult, op1=mult, accum_out=loss_sb,
        )
        po = singles.tile([P, N], F32)
        nc.scalar.activation(
            out=po, in_=preds_buf,
            func=mybir.ActivationFunctionType.Identity,
            bias=b_sb, scale=1.0,
        )
        nc.sync.dma_start(out=preds_o[:, :], in_=po)
        nc.sync.dma_start(out=loss_p[:, :], in_=loss_sb)
    _split_multiwait_instructions(nc)
    return nc


_NC = None


def _get_nc():
    global _NC
    if _NC is None:
        _NC = _build_nc()
    return _NC


def _make_in_maps(outputs, hidden, W, b):
    outputs = np.ascontiguousarray(np.asarray(outputs, dtype=np.float32))
    hidden = np.ascontiguousarray(np.asarray(hidden, dtype=np.float32))
    W = np.ascontiguousarray(np.asarray(W, dtype=np.float32))
    b = np.asarray(b, dtype=np.float32)

    brep_np = np.full((P, 1), b[0], dtype=np.float32)
    # outputs.T flattened s-major matches preds.reshape(-1); bias folded in so
    # the device loss pass is a single subtract.
    ot = np.ascontiguousarray(outputs.T).reshape(-1) - b[0]          # [S*B]

    in_maps = []
    for c in range(NCORES):
        xh_np = hidden[c * S_SH:(c + 1) * S_SH].reshape(P, N * H)
        ot_np = np.ascontiguousarray(ot[c * R:(c + 1) * R].reshape(P, N))
        in_maps.append({"xh": xh_np, "wvec": W, "otadj": ot_np, "brep": brep_np})
    return in_maps


def run(outputs, hidden, W, b, **spmd_kwargs):
    """Run the SPMD kernel; returns ((loss, preds), BassKernelResults)."""
    nc = _get_nc()
    in_maps = _make_in_maps(outputs, hidden, W, b)
    res = run_bass_kernel_spmd(nc, in_maps, list(range(NCORES)), **spmd_kwargs)
    preds = np.concatenate(
        [np.asarray(res.results[c]["preds_o"], dtype=np.float32).reshape(-1)
         for c in range(NCORES)]
    )
    loss = np.float32(
        sum(float(np.asarray(res.results[c]["loss_p"]).sum()) for c in range(NCORES))
    )
    return (loss, preds), res


def kernel(outputs, hidden, W, b):
    (loss, preds), _ = run(outputs, hidden, W, b)
    return loss, preds


if __name__ == "__main__":
    rng = np.random.default_rng(0)
    outputs = rng.standard_normal((B, S), dtype=np.float32)
    hidden = rng.standard_normal((S, B, H), dtype=np.float32)
    W = (rng.standard_normal((1, H), dtype=np.float32)
         * np.float32(np.sqrt(2.0 / (H + 1))))
    b = np.zeros((1,), dtype=np.float32)
    loss, preds = kernel(outputs=outputs, hidden=hidden, W=W, b=b)
    ref_preds = hidden.reshape(-1, H).astype(np.float64) @ W[0].astype(np.float64) + float(b[0])
    ref_loss = np.sum((ref_preds - outputs.T.reshape(-1).astype(np.float64)) ** 2)
    print("preds rel err:", np.abs(preds - ref_preds).max() / np.abs(ref_preds).max())
    print("loss rel err:", abs(float(loss) - ref_loss) / abs(ref_loss))


# revision 12
# speedup vs baseline: 1.1825x; 1.1825x over previous
"""Trainium2 Bass kernel for nn_Decoder: preds = hidden @ W + b, loss = sum((preds - outputs.T)^2).

Full-input contract: kernel(**inputs) takes the unsharded inputs and returns
(loss, preds_flat), matching the reference. Internally shards hidden/outputs
along seq_len across 8 NeuronCores (data-parallel), runs one SPMD Bass kernel,
and combines partial results on the host.

Per-core device program (memory-bound: streams its 32 MiB hidden shard once):
  - hidden shard viewed as [128 partitions, 256 rows x 256 floats] so every
    DMA moves 8 KiB contiguous per partition.
  - the 256-long dot products are split across three engines per 8-row tile:
    A_DVE rows fused on DVE (scalar_tensor_tensor + accumulator), the rest
    multiplied on GpSimd, with K_RED rows reduced by one segmented DVE
    tensor_reduce and R_ACT rows by ACT activation+accumulate.
  - loss partials reduced to [128,1] per core on-device; host sums 8x128 values.
"""

import sys

for _p in ("/opt/trn_rl_repo",):
    if _p not in sys.path:
        sys.path.insert(0, _p)

from contextlib import ExitStack

import numpy as np

import concourse.bass as bass
import concourse.tile as tile
from concourse import mybir
from concourse.bass_utils import run_bass_kernel_spmd

S, B, H = 4096, 64, 256
NCORES = 8
S_SH = S // NCORES          # 512 seq positions per core
R = S_SH * B                # 32768 rows per core
P = 128                     # SBUF partitions
N = R // P                  # 256 rows per partition
TJ = 8                      # rows (segments of H) per partition per tile
NT = N // TJ                # 32 tiles per core

# per-tile split of the 8 row-segments across engines. GpSimd is unusable
# here: it shares SBUF ports with DVE, so concurrent gpsimd work ~3x-slows
# DVE ops (measured 422ns -> 1505ns STT).
A_DVE = 4                   # fused dot on DVE (scalar_tensor_tensor)
K_RED = 0                   # (DVE segmented tensor_reduce path - unused)
R_ACT = TJ - A_DVE - K_RED  # DVE batched mult -> ACT activation+accum
WREP = 8                    # W replicated this many times in SBUF (period for slicing)

F32 = mybir.dt.float32


def _split_multiwait_instructions(nc, max_waits=1):
    """This container's walrus build rejects instructions carrying more than
    one sync-wait command ("Too many sync wait commands" in codegen). Tile's
    end-of-context drain waits on every outstanding proc semaphore in a single
    instruction, so split surplus waits into preceding single-wait drains on
    the same engine."""
    for fn in nc.m.functions:
        for bb in fn.blocks:
            il = bb.instructions
            k = 0
            while k < len(il):
                inst = il[k]
                si = getattr(inst, "sync_info", None)
                if si is not None and si.on_wait and len(si.on_wait) > max_waits:
                    waits = list(si.on_wait)
                    si.on_wait = waits[:max_waits]
                    extra = waits[max_waits:]
                    for i in range(0, len(extra), max_waits):
                        d = mybir.InstDrain(
                            name=f"{inst.name}_wsplit{i}", ins=[], outs=[]
                        )
                        d.engine = inst.engine
                        d.sync_info = mybir.SyncInfo(
                            on_wait=extra[i:i + max_waits], on_update=[]
                        )
                        il.insert(k, d)
                        k += 1
                k += 1


def _build_nc():
    nc = bass.Bass()
    xh = nc.declare_dram_parameter("xh", [P, N * H], F32, isOutput=False)
    wvec = nc.declare_dram_parameter("wvec", [1, H], F32, isOutput=False)
    otadj = nc.declare_dram_parameter("otadj", [P, N], F32, isOutput=False)
    brep = nc.declare_dram_parameter("brep", [P, 1], F32, isOutput=False)
    preds_o = nc.declare_dram_parameter("preds_o", [P, N], F32, isOutput=True)
    loss_p = nc.declare_dram_parameter("loss_p", [P, 1], F32, isOutput=True)

    mult = mybir.AluOpType.mult
    subtract = mybir.AluOpType.subtract
    add = mybir.AluOpType.add

    with tile.TileContext(nc) as tc, ExitStack() as ctx:
        singles = ctx.enter_context(tc.tile_pool(name="singles", bufs=1))
        xpool = ctx.enter_context(tc.tile_pool(name="xp", bufs=8))
        prodp = ctx.enter_context(tc.tile_pool(name="pp", bufs=4))
        scrp = ctx.enter_context(tc.tile_pool(name="sc", bufs=2))

        # W broadcast to all partitions, then replicated TJ times along free
        # (doubling copies) so any aligned 256-slice of w_sb is W.
        w_sb = singles.tile([P, WREP * H], F32)
        nc.sync.dma_start(out=w_sb[:, 0:H], in_=wvec[:, :].to_broadcast([P, H]))
        rep = H
        while rep < WREP * H:
            nc.vector.tensor_copy(w_sb[:, rep:2 * rep], w_sb[:, 0:rep])
            rep *= 2

        ot_sb = singles.tile([P, N], F32)
        b_sb = singles.tile([P, 1], F32)
        preds_buf = singles.tile([P, N], F32)

        GP = K_RED + R_ACT  # segments multiplied on DVE for the ACT path
        for t in range(NT):
            x = xpool.tile([P, TJ * H], F32)
            nc.sync.dma_start(out=x, in_=xh[:, t * TJ * H:(t + 1) * TJ * H])
            c0 = t * TJ
            # multiply for the ACT segments FIRST so ACT's dependency clears
            # while DVE continues with its own fused segments.
            prod = prodp.tile([P, GP * H], F32)
            seg = A_DVE
            off = 0
            while seg < TJ:
                cn = min(WREP - (seg % WREP), TJ - seg)
                nc.vector.tensor_mul(
                    prod[:, off * H:(off + cn) * H],
                    x[:, seg * H:(seg + cn) * H],
                    w_sb[:, (seg % WREP) * H:((seg % WREP) + cn) * H],
                )
                seg += cn
                off += cn
            for j in range(A_DVE):
                sc = scrp.tile([P, H], F32)
                nc.vector.scalar_tensor_tensor(
                    out=sc,
                    in0=x[:, j * H:(j + 1) * H],
                    scalar=1.0,
                    in1=w_sb[:, :H],
                    op0=mult,
                    op1=mult,
                    accum_out=preds_buf[:, c0 + j:c0 + j + 1],
                )
            if K_RED:
                nc.vector.tensor_reduce(
                    out=preds_buf[:, c0 + A_DVE:c0 + A_DVE + K_RED],
                    in_=prod[:, 0:K_RED * H].rearrange("p (k h) -> p k h", k=K_RED),
                    axis=mybir.AxisListType.X,
                    op=add,
                )
            for j in range(R_ACT):
                asc = scrp.tile([P, H], F32)
                nc.scalar.activation(
                    out=asc,
                    in_=prod[:, (K_RED + j) * H:(K_RED + j + 1) * H],
                    func=mybir.ActivationFunctionType.Copy,
                    accum_out=preds_buf[:, c0 + A_DVE + K_RED + j:
                                        c0 + A_DVE + K_RED + j + 1],
                )

        nc.sync.dma_start(out=ot_sb, in_=otadj[:, :])
        nc.sync.dma_start(out=b_sb, in_=brep[:, :])
        d = singles.tile([P, N], F32)
        nc.vector.tensor_tensor(out=d, in0=preds_buf, in1=ot_sb, op=subtract)
        dsc = singles.tile([P, N], F32)
        loss_sb = singles.tile([P, 1], F32)
        nc.vector.scalar_tensor_tensor(
            out=dsc, in0=d, scalar=1.0, in1=d,
            op0=mult, op1=mult, accum_out=loss_sb,
        )
        # bias is added to preds on the host (loss already folds it via otadj)
        nc.sync.dma_start(out=preds_o[:, :], in_=preds_buf)
        nc.sync.dma_start(out=loss_p[:, :], in_=loss_sb)
    _split_multiwait_instructions(nc)
    return nc


_NC = None


def _get_nc():
    global _NC
    if _NC is None:
        _NC = _build_nc()
    return _NC


def _make_in_maps(outputs, hidden, W, b):
    outputs = np.ascontiguousarray(np.asarray(outputs, dtype=np.float32))
    hidden = np.ascontiguousarray(np.asarray(hidden, dtype=np.float32))
    W = np.ascontiguousarray(np.asarray(W, dtype=np.float32))
    b = np.asarray(b, dtype=np.float32)

    brep_np = np.full((P, 1), b[0], dtype=np.float32)
    # outputs.T flattened s-major matches preds.reshape(-1); bias folded in so
    # the device loss pass is a single subtract.
    ot = np.ascontiguousarray(outputs.T).reshape(-1) - b[0]          # [S*B]

    in_maps = []
    for c in range(NCORES):
        xh_np = hidden[c * S_SH:(c + 1) * S_SH].reshape(P, N * H)
        ot_np = np.ascontiguousarray(ot[c * R:(c + 1) * R].reshape(P, N))
        in_maps.append({"xh": xh_np, "wvec": W, "otadj": ot_np, "brep": brep_np})
    return in_maps


def run(outputs, hidden, W, b, **spmd_kwargs):
    """Run the SPMD kernel; returns ((loss, preds), BassKernelResults)."""
    nc = _get_nc()
    in_maps = _make_in_maps(outputs, hidden, W, b)
    res = run_bass_kernel_spmd(nc, in_maps, list(range(NCORES)), **spmd_kwargs)
    preds = np.concatenate(
        [np.asarray(res.results[c]["preds_o"], dtype=np.float32).reshape(-1)
         for c in range(NCORES)]
    )
    bval = np.float32(np.asarray(b, dtype=np.float32).reshape(-1)[0])
    if bval != 0.0:
        preds = preds + bval
    loss = np.float32(
        sum(float(np.asarray(res.results[c]["loss_p"]).sum()) for c in range(NCORES))
    )
    return (loss, preds), res


def kernel(outputs, hidden, W, b):
    (loss, preds), _ = run(outputs, hidden, W, b)
    return loss, preds


if __name__ == "__main__":
    rng = np.random.default_rng(0)
    outputs = rng.standard_normal((B, S), dtype=np.float32)
    hidden = rng.standard_normal((S, B, H), dtype=np.float32)
    W = (rng.standard_normal((1, H), dtype=np.float32)
         * np.float32(np.sqrt(2.0 / (H + 1))))
    b = np.zeros((1,), dtype=np.float32)
    loss, preds = kernel(outputs=outputs, hidden=hidden, W=W, b=b)
    ref_preds = hidden.reshape(-1, H).astype(np.float64) @ W[0].astype(np.float64) + float(b[0])
    ref_loss = np.sum((ref_preds - outputs.T.reshape(-1).astype(np.float64)) ** 2)
    print("preds rel err:", np.abs(preds - ref_preds).max() / np.abs(ref_preds).max())
    print("loss rel err:", abs(float(loss) - ref_loss) / abs(ref_loss))


# revision 13
# speedup vs baseline: 1.2824x; 1.0845x over previous
"""Trainium2 Bass kernel for nn_Decoder: preds = hidden @ W + b, loss = sum((preds - outputs.T)^2).

Full-input contract: kernel(**inputs) takes the unsharded inputs and returns
(loss, preds_flat), matching the reference. Internally shards hidden/outputs
along seq_len across 8 NeuronCores (data-parallel), runs one SPMD Bass kernel,
and combines partial results on the host.

Per-core device program (memory-bound: streams its 32 MiB hidden shard once):
  - hidden shard viewed as [128 partitions, 256 rows x 256 floats] so every
    DMA moves 8 KiB contiguous per partition.
  - the 256-long dot products are split across three engines per 8-row tile:
    A_DVE rows fused on DVE (scalar_tensor_tensor + accumulator), the rest
    multiplied on GpSimd, with K_RED rows reduced by one segmented DVE
    tensor_reduce and R_ACT rows by ACT activation+accumulate.
  - loss partials reduced to [128,1] per core on-device; host sums 8x128 values.
"""

import sys

for _p in ("/opt/trn_rl_repo",):
    if _p not in sys.path:
        sys.path.insert(0, _p)

from contextlib import ExitStack

import numpy as np

import concourse.bass as bass
import concourse.tile as tile
from concourse import mybir
from concourse.bass_utils import run_bass_kernel_spmd

S, B, H = 4096, 64, 256
NCORES = 8
S_SH = S // NCORES          # 512 seq positions per core
R = S_SH * B                # 32768 rows per core
P = 128                     # SBUF partitions
N = R // P                  # 256 rows per partition
TJ = 8                      # rows (segments of H) per partition per tile
NT = N // TJ                # 32 tiles per core

# per-tile split of the 8 row-segments across engines. GpSimd is unusable
# here: it shares SBUF ports with DVE, so concurrent gpsimd work ~3x-slows
# DVE ops (measured 422ns -> 1505ns STT).
A_DVE = 4                   # fused dot on DVE (scalar_tensor_tensor)
K_RED = 0                   # (DVE segmented tensor_reduce path - unused)
R_ACT = TJ - A_DVE - K_RED  # DVE batched mult -> ACT activation+accum
WREP = 8                    # W replicated this many times in SBUF (period for slicing)

F32 = mybir.dt.float32


def _split_multiwait_instructions(nc, max_waits=1):
    """This container's walrus build rejects instructions carrying more than
    one sync-wait command ("Too many sync wait commands" in codegen). Tile's
    end-of-context drain waits on every outstanding proc semaphore in a single
    instruction, so split surplus waits into preceding single-wait drains on
    the same engine."""
    for fn in nc.m.functions:
        for bb in fn.blocks:
            il = bb.instructions
            k = 0
            while k < len(il):
                inst = il[k]
                si = getattr(inst, "sync_info", None)
                if si is not None and si.on_wait and len(si.on_wait) > max_waits:
                    waits = list(si.on_wait)
                    si.on_wait = waits[:max_waits]
                    extra = waits[max_waits:]
                    for i in range(0, len(extra), max_waits):
                        d = mybir.InstDrain(
                            name=f"{inst.name}_wsplit{i}", ins=[], outs=[]
                        )
                        d.engine = inst.engine
                        d.sync_info = mybir.SyncInfo(
                            on_wait=extra[i:i + max_waits], on_update=[]
                        )
                        il.insert(k, d)
                        k += 1
                k += 1


def _build_nc():
    nc = bass.Bass()
    xh = nc.declare_dram_parameter("xh", [P, N * H], F32, isOutput=False)
    wvec = nc.declare_dram_parameter("wvec", [1, H], F32, isOutput=False)
    otadj = nc.declare_dram_parameter("otadj", [P, N], F32, isOutput=False)
    # single output: columns 0..N-1 = preds (pre-bias), column N = loss partials
    preds_o = nc.declare_dram_parameter("preds_o", [P, N + 1], F32, isOutput=True)

    mult = mybir.AluOpType.mult
    subtract = mybir.AluOpType.subtract
    add = mybir.AluOpType.add

    with tile.TileContext(nc) as tc, ExitStack() as ctx:
        singles = ctx.enter_context(tc.tile_pool(name="singles", bufs=1))
        xpool = ctx.enter_context(tc.tile_pool(name="xp", bufs=8))
        prodp = ctx.enter_context(tc.tile_pool(name="pp", bufs=4))
        scrp = ctx.enter_context(tc.tile_pool(name="sc", bufs=2))

        # W broadcast to all partitions, then replicated TJ times along free
        # (doubling copies) so any aligned 256-slice of w_sb is W.
        w_sb = singles.tile([P, WREP * H], F32)
        nc.sync.dma_start(out=w_sb[:, 0:H], in_=wvec[:, :].to_broadcast([P, H]))
        rep = H
        while rep < WREP * H:
            nc.vector.tensor_copy(w_sb[:, rep:2 * rep], w_sb[:, 0:rep])
            rep *= 2

        ot_sb = singles.tile([P, N], F32)
        preds_buf = singles.tile([P, N + 1], F32)

        GP = K_RED + R_ACT  # segments multiplied on DVE for the ACT path
        for t in range(NT):
            x = xpool.tile([P, TJ * H], F32)
            nc.sync.dma_start(out=x, in_=xh[:, t * TJ * H:(t + 1) * TJ * H])
            c0 = t * TJ
            # multiply for the ACT segments FIRST so ACT's dependency clears
            # while DVE continues with its own fused segments.
            prod = prodp.tile([P, GP * H], F32)
            seg = A_DVE
            off = 0
            while seg < TJ:
                cn = min(WREP - (seg % WREP), TJ - seg)
                nc.vector.tensor_mul(
                    prod[:, off * H:(off + cn) * H],
                    x[:, seg * H:(seg + cn) * H],
                    w_sb[:, (seg % WREP) * H:((seg % WREP) + cn) * H],
                )
                seg += cn
                off += cn
            for j in range(A_DVE):
                sc = scrp.tile([P, H], F32)
                nc.vector.scalar_tensor_tensor(
                    out=sc,
                    in0=x[:, j * H:(j + 1) * H],
                    scalar=1.0,
                    in1=w_sb[:, :H],
                    op0=mult,
                    op1=mult,
                    accum_out=preds_buf[:, c0 + j:c0 + j + 1],
                )
            if K_RED:
                nc.vector.tensor_reduce(
                    out=preds_buf[:, c0 + A_DVE:c0 + A_DVE + K_RED],
                    in_=prod[:, 0:K_RED * H].rearrange("p (k h) -> p k h", k=K_RED),
                    axis=mybir.AxisListType.X,
                    op=add,
                )
            for j in range(R_ACT):
                asc = scrp.tile([P, H], F32)
                nc.scalar.activation(
                    out=asc,
                    in_=prod[:, (K_RED + j) * H:(K_RED + j + 1) * H],
                    func=mybir.ActivationFunctionType.Copy,
                    accum_out=preds_buf[:, c0 + A_DVE + K_RED + j:
                                        c0 + A_DVE + K_RED + j + 1],
                )

        nc.sync.dma_start(out=ot_sb, in_=otadj[:, :])
        d = singles.tile([P, N], F32)
        nc.vector.tensor_tensor(out=d, in0=preds_buf[:, 0:N], in1=ot_sb, op=subtract)
        dsc = singles.tile([P, N], F32)
        nc.vector.scalar_tensor_tensor(
            out=dsc, in0=d, scalar=1.0, in1=d,
            op0=mult, op1=mult, accum_out=preds_buf[:, N:N + 1],
        )
        # bias is added to preds on the host (loss already folds it via otadj);
        # loss rides in column N of the one contiguous store (a separate
        # [128,1] store pays ~10us of straggling 4B-packet sem completions).
        nc.sync.dma_start(out=preds_o[:, :], in_=preds_buf)
    _split_multiwait_instructions(nc)
    return nc


_NC = None


def _get_nc():
    global _NC
    if _NC is None:
        _NC = _build_nc()
    return _NC


def _make_in_maps(outputs, hidden, W, b):
    outputs = np.ascontiguousarray(np.asarray(outputs, dtype=np.float32))
    hidden = np.ascontiguousarray(np.asarray(hidden, dtype=np.float32))
    W = np.ascontiguousarray(np.asarray(W, dtype=np.float32))
    b = np.asarray(b, dtype=np.float32)

    # outputs.T flattened s-major matches preds.reshape(-1); bias folded in so
    # the device loss pass is a single subtract.
    ot = np.ascontiguousarray(outputs.T).reshape(-1) - b[0]          # [S*B]

    in_maps = []
    for c in range(NCORES):
        xh_np = hidden[c * S_SH:(c + 1) * S_SH].reshape(P, N * H)
        ot_np = np.ascontiguousarray(ot[c * R:(c + 1) * R].reshape(P, N))
        in_maps.append({"xh": xh_np, "wvec": W, "otadj": ot_np})
    return in_maps


def run(outputs, hidden, W, b, **spmd_kwargs):
    """Run the SPMD kernel; returns ((loss, preds), BassKernelResults)."""
    nc = _get_nc()
    in_maps = _make_in_maps(outputs, hidden, W, b)
    res = run_bass_kernel_spmd(nc, in_maps, list(range(NCORES)), **spmd_kwargs)
    outs = [np.asarray(res.results[c]["preds_o"], dtype=np.float32) for c in range(NCORES)]
    preds = np.concatenate([o[:, :N].reshape(-1) for o in outs])
    bval = np.float32(np.asarray(b, dtype=np.float32).reshape(-1)[0])
    if bval != 0.0:
        preds = preds + bval
    loss = np.float32(sum(float(o[:, N].sum()) for o in outs))
    return (loss, preds), res


def kernel(outputs, hidden, W, b):
    (loss, preds), _ = run(outputs, hidden, W, b)
    return loss, preds


if __name__ == "__main__":
    rng = np.random.default_rng(0)
    outputs = rng.standard_normal((B, S), dtype=np.float32)
    hidden = rng.standard_normal((S, B, H), dtype=np.float32)
    W = (rng.standard_normal((1, H), dtype=np.float32)
         * np.float32(np.sqrt(2.0 / (H + 1))))
    b = np.zeros((1,), dtype=np.float32)
    loss, preds = kernel(outputs=outputs, hidden=hidden, W=W, b=b)
    ref_preds = hidden.reshape(-1, H).astype(np.float64) @ W[0].astype(np.float64) + float(b[0])
    ref_loss = np.sum((ref_preds - outputs.T.reshape(-1).astype(np.float64)) ** 2)
    print("preds rel err:", np.abs(preds - ref_preds).max() / np.abs(ref_preds).max())
    print("loss rel err:", abs(float(loss) - ref_loss) / abs(ref_loss))
